# revision 1
# baseline (speedup 1.0000x reference)
"""Trainium2 Bass kernel for nn_HCF_module (SC2 NMS/registration pipeline).

Sharding: 512 seeds split across 8 NeuronCores (64 seeds/core, keypoints
replicated). Device launches (SPMD on cores 0-7 via run_bass_kernel_spmd):
  L1: per-seed top-200 extraction over SC2 rows (exact jax top_k tie order
      via DVE max/max_index/match_replace rounds)
  L2-L5: filter stages k=200/100/50/25 -> per-seed SC2 consistency scores
      (elementwise pairwise-d2 + sqrt-free hard-bit test + row-0 product)
  L6: fitness counts (rigid-transform inlier counting over all 2048 pts)
Host glue between launches: index gathers, final k=12 Kabsch (f32).
"""
import numpy as np

F32 = np.float32
T2 = F32(0.1) * F32(0.1)            # 0.010000000707...
TWO_T2 = F32(2.0) * T2
T4 = T2 * T2
NCORES = 8
SEEDS = 512
SPC = SEEDS // NCORES               # seeds per core
NPTS = 2048

_programs = {}
_launch_wall = []


def _mk_bass():
    import concourse.bass as bass
    return bass.Bass("TRN2", target_bir_lowering=False)


def _prog_topk():
    """[128, 1024] f32 (row 2s+h = seed s, half h) -> top-136 values+idx per half.
    Outputs ym [128,136] f32, yi [128,136] uint32 (local idx in half)."""
    import concourse.mybir as mybir
    nc = _mk_bass()
    P, HN, R = 128, NPTS // 2, 17
    x = nc.dram_tensor("x", [P, HN], mybir.dt.float32, kind="ExternalInput")
    ym = nc.dram_tensor("ym", [P, 8 * R], mybir.dt.float32, kind="ExternalOutput")
    yi = nc.dram_tensor("yi", [P, 8 * R], mybir.dt.uint32, kind="ExternalOutput")
    ctx = nc.ctx
    t = ctx.enter_context(nc.sbuf_tensor([P, HN], mybir.dt.float32))
    m8 = ctx.enter_context(nc.sbuf_tensor([P, 8 * R], mybir.dt.float32))
    i8 = ctx.enter_context(nc.sbuf_tensor([P, 8 * R], mybir.dt.uint32))
    dma_sem = ctx.enter_context(nc.semaphore())
    vsem = ctx.enter_context(nc.semaphore())
    with nc.Block() as block:
        @block.gpsimd
        def _(gpsimd):
            gpsimd.dma_start(t[:, :], x[:, :]).then_inc(dma_sem, 16)
            gpsimd.wait_ge(vsem, 3 * R)
            gpsimd.dma_start(ym[:, :], m8[:, :]).then_inc(dma_sem, 16)
            gpsimd.dma_start(yi[:, :], i8[:, :]).then_inc(dma_sem, 16)
            gpsimd.wait_ge(dma_sem, 48)

        @block.vector
        def _(vector):
            vector.wait_ge(dma_sem, 16)
            n = 0
            for r in range(R):
                sl = slice(r * 8, (r + 1) * 8)
                nc.vector.max(out=m8[:, sl], in_=t[:, :]).then_inc(vsem, 1)
                n += 1
                vector.wait_ge(vsem, n)
                nc.vector.max_index(out=i8[:, sl], in_max=m8[:, sl],
                                    in_values=t[:, :]).then_inc(vsem, 1)
                n += 1
                nc.vector.match_replace(out=t[:, :], in_to_replace=m8[:, sl],
                                        in_values=t[:, :], imm_value=-1e30).then_inc(vsem, 1)
                n += 1
                vector.wait_ge(vsem, n)
    return nc


def _prog_sc2(k):
    """gx,gy [SPC, 3*k] f32 (c-major: x|y|z rows) -> sc2 [SPC, k] f32."""
    import concourse.mybir as mybir
    from concourse.alu_op_type import AluOpType as OP
    nc = _mk_bass()
    gx = nc.dram_tensor("gx", [SPC, 3 * k], mybir.dt.float32, kind="ExternalInput")
    gy = nc.dram_tensor("gy", [SPC, 3 * k], mybir.dt.float32, kind="ExternalInput")
    out = nc.dram_tensor("sc2", [SPC, k], mybir.dt.float32, kind="ExternalOutput")
    ctx = nc.ctx
    B = 20 if k % 20 == 0 else 25  # k=200/100 -> 20, k=50/25 -> 25
    if k % B:
        B = 5
    assert k % B == 0
    tx = ctx.enter_context(nc.sbuf_tensor([SPC, 3 * k], mybir.dt.float32))
    ty = ctx.enter_context(nc.sbuf_tensor([SPC, 3 * k], mybir.dt.float32))
    dxs = ctx.enter_context(nc.sbuf_tensor([SPC, B * 3 * k], mybir.dt.float32))
    d2a = ctx.enter_context(nc.sbuf_tensor([SPC, B * k], mybir.dt.float32))
    d2b = ctx.enter_context(nc.sbuf_tensor([SPC, B * k], mybir.dt.float32))
    q = ctx.enter_context(nc.sbuf_tensor([SPC, B * k], mybir.dt.float32))
    p = ctx.enter_context(nc.sbuf_tensor([SPC, B * k], mybir.dt.float32))
    hard = ctx.enter_context(nc.sbuf_tensor([SPC, B * k], mybir.dt.float32))
    scr = ctx.enter_context(nc.sbuf_tensor([SPC, B * k], mybir.dt.float32))
    h0 = ctx.enter_context(nc.sbuf_tensor([SPC, k], mybir.dt.float32))
    sc2 = ctx.enter_context(nc.sbuf_tensor([SPC, k], mybir.dt.float32))
    dma_sem = ctx.enter_context(nc.semaphore())
    vsem = ctx.enter_context(nc.semaphore())
    nb = k // B
    vcount = [0]

    veng = [None]

    def _fence():
        veng[0].wait_ge(vsem, vcount[0])

    def tt(out_ap, a_ap, b_ap, op):
        nc.vector.tensor_tensor(out=out_ap, in0=a_ap, in1=b_ap, op=op).then_inc(vsem, 1)
        vcount[0] += 1
        _fence()

    def ts(out_ap, a_ap, s1, op0, s2=None, op1=None):
        if op1 is None:
            nc.vector.tensor_scalar(out_ap, a_ap, s1, None, op0).then_inc(vsem, 1)
        else:
            nc.vector.tensor_scalar(out_ap, a_ap, s1, s2, op0, op1).then_inc(vsem, 1)
        vcount[0] += 1
        _fence()

    with nc.Block() as block:
        @block.vector
        def _(vector):
            veng[0] = vector
            vector.wait_ge(dma_sem, 32)
            for bi in range(nb):
                a0 = bi * B
                for (src_t, dst) in ((tx, d2a), (ty, d2b)):
                    v3 = src_t[:, :].rearrange("p (c b) -> p c b", c=3)      # [p,3,k]
                    rows4 = v3.unsqueeze(1).to_broadcast([SPC, B, 3, k])
                    cols4 = v3[:, :, a0:a0 + B].transpose([0, 2, 1]).unsqueeze(3).to_broadcast([SPC, B, 3, k])
                    dx4 = dxs[:, :].rearrange("p (a c b) -> p a c b", a=B, c=3)
                    tt(dx4, rows4, cols4, OP.subtract)
                    tt(dxs[:, :], dxs[:, :], dxs[:, :], OP.mult)
                    d2v = dst[:, :].rearrange("p (a b) -> p a b", a=B)
                    tt(d2v, dx4[:, :, 0, :], dx4[:, :, 1, :], OP.add)
                    tt(d2v, d2v, dx4[:, :, 2, :], OP.add)
                tt(q[:, :], d2a[:, :], d2b[:, :], OP.add)
                tt(p[:, :], d2a[:, :], d2b[:, :], OP.subtract)
                tt(p[:, :], p[:, :], p[:, :], OP.mult)
                ts(scr[:, :], q[:, :], float(TWO_T2), OP.mult, float(T4), OP.subtract)
                tt(hard[:, :], p[:, :], scr[:, :], OP.is_lt)
                ts(scr[:, :], q[:, :], float(T2), OP.is_lt)
                tt(hard[:, :], hard[:, :], scr[:, :], OP.max)
                if bi == 0:
                    nc.vector.tensor_copy(h0[:, :], hard[:, :k]).then_inc(vsem, 1)
                    vcount[0] += 1
                    _fence()
                hv = hard[:, :].rearrange("p (a b) -> p a b", a=B)
                h0c = h0[:, a0:a0 + B].unsqueeze(2).to_broadcast([SPC, B, k])
                tt(hv, hv, h0c, OP.mult)
                hT = hv.transpose([0, 2, 1])                                  # [p,k,a]
                if bi == 0:
                    nc.vector.tensor_reduce(out=sc2[:, :], in_=hT, axis=mybir.AxisListType.X,
                                            op=OP.add).then_inc(vsem, 1)
                    vcount[0] += 1
                    _fence()
                else:
                    nc.vector.tensor_reduce(out=scr[:, :k], in_=hT, axis=mybir.AxisListType.X,
                                            op=OP.add).then_inc(vsem, 1)
                    vcount[0] += 1
                    _fence()
                    tt(sc2[:, :], sc2[:, :], scr[:, :k], OP.add)

        @block.gpsimd
        def _(gpsimd):
            gpsimd.dma_start(tx[:, :], gx[:, :]).then_inc(dma_sem, 16)
            gpsimd.dma_start(ty[:, :], gy[:, :]).then_inc(dma_sem, 16)
            gpsimd.wait_ge(vsem, vcount[0])
            gpsimd.dma_start(out[:, :], sc2[:, :]).then_inc(dma_sem, 16)
            gpsimd.wait_ge(dma_sem, 48)
    return nc


def _prog_fitness():
    """srcb,tgtb [128, 3*1024] (c-major halves), r12 [128, 12] -> cnt [128, 1]."""
    import concourse.mybir as mybir
    from concourse.alu_op_type import AluOpType as OP
    nc = _mk_bass()
    P, HN = 128, NPTS // 2
    srcb = nc.dram_tensor("srcb", [P, 3 * HN], mybir.dt.float32, kind="ExternalInput")
    tgtb = nc.dram_tensor("tgtb", [P, 3 * HN], mybir.dt.float32, kind="ExternalInput")
    r12 = nc.dram_tensor("r12", [P, 12], mybir.dt.float32, kind="ExternalInput")
    cnt = nc.dram_tensor("cnt", [P, 1], mybir.dt.float32, kind="ExternalOutput")
    ctx = nc.ctx
    ts_ = ctx.enter_context(nc.sbuf_tensor([P, 3 * HN], mybir.dt.float32))
    tt_ = ctx.enter_context(nc.sbuf_tensor([P, 3 * HN], mybir.dt.float32))
    tr = ctx.enter_context(nc.sbuf_tensor([P, 12], mybir.dt.float32))
    acc = ctx.enter_context(nc.sbuf_tensor([P, HN], mybir.dt.float32))
    dc = ctx.enter_context(nc.sbuf_tensor([P, 3 * HN], mybir.dt.float32))
    l2s = ctx.enter_context(nc.sbuf_tensor([P, HN], mybir.dt.float32))
    sq = ctx.enter_context(nc.sbuf_tensor([P, HN], mybir.dt.float32))
    ccol = ctx.enter_context(nc.sbuf_tensor([P, 1], mybir.dt.float32))
    dma_sem = ctx.enter_context(nc.semaphore())
    vsem = ctx.enter_context(nc.semaphore())
    vcount = [0]

    with nc.Block() as block:
        @block.vector
        def _(vector):
            def fence():
                vector.wait_ge(vsem, vcount[0])

            def emit(inst):
                inst.then_inc(vsem, 1)
                vcount[0] += 1
                fence()

            vector.wait_ge(dma_sem, 48)
            xv = ts_[:, :].rearrange("p (c b) -> p c b", c=3)
            yvv = tt_[:, :].rearrange("p (c b) -> p c b", c=3)
            dv = dc[:, :].rearrange("p (c b) -> p c b", c=3)
            for c in range(3):
                emit(nc.vector.tensor_scalar(acc[:, :], xv[:, 0, :], tr[:, 4 * c:4 * c + 1],
                                             tr[:, 4 * c + 3:4 * c + 4], OP.mult, OP.add))
                for j in (1, 2):
                    emit(nc.vector.scalar_tensor_tensor(
                        out=acc[:, :], in0=xv[:, j, :], scalar=tr[:, 4 * c + j:4 * c + j + 1],
                        in1=acc[:, :], op0=OP.mult, op1=OP.add))
                emit(nc.vector.tensor_tensor(out=dv[:, c, :], in0=acc[:, :], in1=yvv[:, c, :],
                                             op=OP.subtract))
            emit(nc.vector.tensor_tensor(out=l2s[:, :], in0=dv[:, 0, :], in1=dv[:, 0, :], op=OP.mult))
            emit(nc.vector.tensor_tensor(out=sq[:, :], in0=dv[:, 1, :], in1=dv[:, 1, :], op=OP.mult))
            emit(nc.vector.tensor_tensor(out=l2s[:, :], in0=l2s[:, :], in1=sq[:, :], op=OP.add))
            emit(nc.vector.tensor_tensor(out=sq[:, :], in0=dv[:, 2, :], in1=dv[:, 2, :], op=OP.mult))
            emit(nc.vector.tensor_tensor(out=l2s[:, :], in0=l2s[:, :], in1=sq[:, :], op=OP.add))
            emit(nc.vector.tensor_scalar(sq[:, :], l2s[:, :], float(T2), None, OP.is_lt))
            emit(nc.vector.tensor_reduce(out=ccol[:, :], in_=sq[:, :], axis=mybir.AxisListType.X,
                                         op=OP.add))

        @block.gpsimd
        def _(gpsimd):
            gpsimd.dma_start(ts_[:, :], srcb[:, :]).then_inc(dma_sem, 16)
            gpsimd.dma_start(tt_[:, :], tgtb[:, :]).then_inc(dma_sem, 16)
            gpsimd.dma_start(tr[:, :], r12[:, :]).then_inc(dma_sem, 16)
            gpsimd.wait_ge(vsem, vcount[0])
            gpsimd.dma_start(cnt[:, :], ccol[:, :]).then_inc(dma_sem, 16)
            gpsimd.wait_ge(dma_sem, 64)
    return nc


def _get_prog(key, builder):
    if key not in _programs:
        _programs[key] = builder()
    return _programs[key]


def _run(nc, in_maps):
    import time
    from concourse.bass_utils import run_bass_kernel_spmd
    last = None
    for attempt in range(3):
        try:
            t0 = time.time()
            res = run_bass_kernel_spmd(nc, in_maps, core_ids=list(range(NCORES)))
            _launch_wall.append(time.time() - t0)
            return res.results
        except Exception as e:  # transient device errors: retry
            last = e
    raise last


# ---------------- host-side math (validated f32 device-grade model) -------------

def _topk_host(vals, kk):
    return np.argsort(-vals, axis=-1, kind='stable')[..., :kk]


def _recip(x):
    return (np.float64(1.0) / x.astype(np.float64)).astype(F32)


def _sqrt32(x):
    return np.sqrt(x.astype(np.float64)).astype(F32)


def _cross3(a, b):
    c0 = (a[..., 1] * b[..., 2]).astype(F32) - (a[..., 2] * b[..., 1]).astype(F32)
    c1 = (a[..., 2] * b[..., 0]).astype(F32) - (a[..., 0] * b[..., 2]).astype(F32)
    c2 = (a[..., 0] * b[..., 1]).astype(F32) - (a[..., 1] * b[..., 0]).astype(F32)
    return np.stack([c0.astype(F32), c1.astype(F32), c2.astype(F32)], -1)


def _eig3(K):
    S = K.shape[0]
    qq = ((K[:, 0, 0] + K[:, 1, 1]).astype(F32) + K[:, 2, 2]).astype(F32) * F32(1 / 3)
    qq = qq.astype(F32)
    K00 = (K[:, 0, 0] - qq).astype(F32); K11 = (K[:, 1, 1] - qq).astype(F32); K22 = (K[:, 2, 2] - qq).astype(F32)
    p1 = ((K[:, 0, 1] ** 2).astype(F32) + (K[:, 0, 2] ** 2).astype(F32) + (K[:, 1, 2] ** 2).astype(F32)).astype(F32)
    p2 = ((K00 ** 2).astype(F32) + (K11 ** 2).astype(F32) + (K22 ** 2).astype(F32) + (F32(2) * p1).astype(F32)).astype(F32)
    p = _sqrt32((p2 * F32(1 / 6)).astype(F32))
    rp = _recip(np.maximum(p, F32(1e-30)))
    B00 = (K00 * rp).astype(F32); B11 = (K11 * rp).astype(F32); B22 = (K22 * rp).astype(F32)
    B01 = (K[:, 0, 1] * rp).astype(F32); B02 = (K[:, 0, 2] * rp).astype(F32); B12 = (K[:, 1, 2] * rp).astype(F32)
    detB = (B00 * ((B11 * B22).astype(F32) - (B12 * B12).astype(F32)).astype(F32)).astype(F32) \
        - (B01 * ((B01 * B22).astype(F32) - (B12 * B02).astype(F32)).astype(F32)).astype(F32) \
        + (B02 * ((B01 * B12).astype(F32) - (B11 * B02).astype(F32)).astype(F32)).astype(F32)
    r = np.clip((detB.astype(F32) * F32(0.5)).astype(F32), F32(-1), F32(1))
    c = np.ones(S, F32)
    for _ in range(6):
        f = ((F32(4) * c * c * c).astype(F32) - (F32(3) * c).astype(F32) - r).astype(F32)
        fp = ((F32(12) * c * c).astype(F32) - F32(3)).astype(F32)
        c = np.clip((c - (f * _recip(np.maximum(fp, F32(1e-6)))).astype(F32)).astype(F32), F32(0.5), F32(1.0))
    s_ = _sqrt32(np.maximum((F32(1) - (c * c).astype(F32)).astype(F32), F32(0)))
    lam1 = (qq + (F32(2) * p * c).astype(F32)).astype(F32)
    cmid = ((F32(-0.5) * c).astype(F32) + (F32(np.sqrt(3) / 2) * s_).astype(F32)).astype(F32)
    lam2 = (qq + (F32(2) * p * cmid).astype(F32)).astype(F32)
    return lam1, lam2


def _eigvec(K, lam):
    A = K.astype(F32).copy()
    for i in range(3):
        A[:, i, i] = (A[:, i, i] - lam).astype(F32)
    r0, r1, r2 = A[:, 0, :], A[:, 1, :], A[:, 2, :]
    c1 = _cross3(r0, r1); c2 = _cross3(r1, r2); c3 = _cross3(r2, r0)
    n1 = (c1 ** 2).sum(-1).astype(F32); n2 = (c2 ** 2).sum(-1).astype(F32); n3 = (c3 ** 2).sum(-1).astype(F32)
    a1 = (n1 >= n2) & (n1 >= n3); a2 = (~a1) & (n2 >= n3); a3 = ~(a1 | a2)
    u = (c1 * a1[:, None] + c2 * a2[:, None] + c3 * a3[:, None]).astype(F32)
    n = (u ** 2).sum(-1).astype(F32)
    return (u * _recip(_sqrt32(np.maximum(n, F32(1e-38))))[:, None]).astype(F32)


def _kabsch(A, B, w):
    S = A.shape[0]
    wsum = w.sum(axis=1, dtype=np.float32)
    rws = _recip((wsum + F32(1e-6)).astype(F32))
    wA = (A * w[:, :, None]).astype(F32); wB = (B * w[:, :, None]).astype(F32)
    cA = (wA.sum(axis=1, dtype=np.float32) * rws[:, None]).astype(F32)
    cB = (wB.sum(axis=1, dtype=np.float32) * rws[:, None]).astype(F32)
    Am = (A - cA[:, None, :]).astype(F32); Bm = (B - cB[:, None, :]).astype(F32)
    wAm = (Am * w[:, :, None]).astype(F32)
    H = np.einsum('ski,skj->sij', wAm, Bm).astype(F32)
    K = np.einsum('sij,skj->sik', H, H).astype(F32)
    lam1, lam2 = _eig3(K)
    u1 = _eigvec(K, lam1)
    u2r = _eigvec(K, lam2)
    dot = (u1 * u2r).sum(-1).astype(F32)
    u2 = (u2r - u1 * dot[:, None]).astype(F32)
    n = (u2 ** 2).sum(-1).astype(F32)
    u2 = (u2 * _recip(_sqrt32(np.maximum(n, F32(1e-38))))[:, None]).astype(F32)
    u3 = _cross3(u1, u2)
    w1 = np.einsum('ski,sk->si', H, u1).astype(F32)
    w2 = np.einsum('ski,sk->si', H, u2).astype(F32)
    v1 = (w1 * _recip(_sqrt32(np.maximum((w1 ** 2).sum(-1).astype(F32), F32(1e-38))))[:, None]).astype(F32)
    v2 = (w2 * _recip(_sqrt32(np.maximum((w2 ** 2).sum(-1).astype(F32), F32(1e-38))))[:, None]).astype(F32)
    v3 = _cross3(v1, v2)
    R = (v1[:, :, None] * u1[:, None, :] + v2[:, :, None] * u2[:, None, :]
         + v3[:, :, None] * u3[:, None, :]).astype(F32)
    t = (cB - np.einsum('sij,sj->si', R, cA).astype(F32)).astype(F32)
    return R, t


def _power_iter(M):
    S, k, _ = M.shape
    v = np.ones((S, k), F32)
    for _ in range(10):
        prod = (M * v[:, None, :]).astype(F32)
        acc = prod[:, :, 0]
        for j in range(1, k):
            acc = (acc + prod[:, :, j]).astype(F32)
        n2 = (acc * acc).astype(F32)
        s2 = n2[:, 0]
        for j in range(1, k):
            s2 = (s2 + n2[:, j]).astype(F32)
        nn_ = _sqrt32(s2)
        v = (acc * _recip((nn_ + F32(1e-6)).astype(F32))[:, None]).astype(F32)
    return v


def _pdist2(pts):
    d = (pts[:, :, None, :] - pts[:, None, :, :]).astype(F32)
    sq = (d * d).astype(F32)
    return ((sq[..., 0] + sq[..., 1]).astype(F32) + sq[..., 2]).astype(F32)


def kernel(SC2_measure, src_keypts, tgt_keypts):
    _launch_wall.clear()
    SC2 = np.ascontiguousarray(SC2_measure[0], dtype=np.float32)      # [512, 2048]
    src = np.ascontiguousarray(src_keypts[0], dtype=np.float32)       # [2048, 3]
    tgt = np.ascontiguousarray(tgt_keypts[0], dtype=np.float32)

    # ---- L1: per-seed top-200 on device (rows split into 2 halves) ----
    nc1 = _get_prog("topk", _prog_topk)
    HN = NPTS // 2
    xh = SC2.reshape(SEEDS, 2, HN).reshape(SEEDS * 2, HN)  # row 2s+h
    in_maps = [{"x": xh[c * 2 * SPC:(c + 1) * 2 * SPC]} for c in range(NCORES)]
    for _try in range(4):
        res = _run(nc1, in_maps)
        vm = np.concatenate([res[c]["ym"] for c in range(NCORES)], axis=0)
        vi = np.concatenate([res[c]["yi"] for c in range(NCORES)], axis=0).astype(np.int64)
        if (vi < HN).all():
            break
    # merge halves: concat [A|B]; stable sort by value desc == jax global order
    NE = vm.shape[1]
    cand_v = np.concatenate([vm[0::2], vm[1::2]], axis=1)            # [512, 2*NE]
    cand_i = np.concatenate([vi[0::2], vi[1::2] + HN], axis=1)
    order = np.argsort(-cand_v, axis=1, kind='stable')[:, :200]
    knn = np.take_along_axis(cand_i, order, axis=1)                  # [512, 200]
    # safety: if any seed's 200th value ties the last extracted value of a
    # half, extraction may be incomplete -> exact host fallback for that seed
    thr = np.take_along_axis(cand_v, order[:, 199:200], axis=1)[:, 0]
    risky = (vm[0::2, NE - 1] >= thr) | (vm[1::2, NE - 1] >= thr)
    for s in np.where(risky)[0]:
        knn[s] = np.argsort(-SC2[s], kind='stable')[:200]
    sknn = src[knn].astype(F32)                                       # [512, 200, 3]
    tknn = tgt[knn].astype(F32)

    # ---- L2-L5: filter stages on device ----
    k = 200
    while k > 15:
        nck = _get_prog(("sc2", k), lambda kk=k: _prog_sc2(kk))
        gxa = np.ascontiguousarray(np.transpose(sknn, (0, 2, 1)).reshape(SEEDS, 3 * k))
        gya = np.ascontiguousarray(np.transpose(tknn, (0, 2, 1)).reshape(SEEDS, 3 * k))
        in_maps = [{"gx": gxa[c * SPC:(c + 1) * SPC], "gy": gya[c * SPC:(c + 1) * SPC]}
                   for c in range(NCORES)]
        for _try in range(4):
            res = _run(nck, in_maps)
            sc2 = np.concatenate([res[c]["sc2"] for c in range(NCORES)], axis=0)
            ok = (sc2 == np.round(sc2)).all() and (sc2 >= 0).all() and (sc2 <= k).all() and (sc2[:, 0] >= 1).all()
            if ok:
                break
        kf = k // 2
        sel = _topk_host(sc2, kf)                                     # ties: pos asc
        sknn = np.take_along_axis(sknn, sel[:, :, None], axis=1)
        tknn = np.take_along_axis(tknn, sel[:, :, None], axis=1)
        k = kf
    # k == 12

    # ---- host: local_sc, power iteration, Kabsch (validated f32 model) ----
    a2 = _pdist2(sknn); b2 = _pdist2(tknn)
    da = _sqrt32(np.maximum(a2, F32(1e-12)))
    db = _sqrt32(np.maximum(b2, F32(1e-12)))
    cross = np.abs((da - db).astype(F32)).astype(F32)
    local_sc = np.maximum(F32(1.0) - ((cross * cross).astype(F32) / T2).astype(F32), F32(0.0)).astype(F32)
    eye = np.eye(12, dtype=F32)
    M = (local_sc * (F32(1.0) - eye)[None]).astype(F32)
    v = _power_iter(M)
    wsum = v[:, 0].copy()
    for j in range(1, 12):
        wsum = (wsum + v[:, j]).astype(F32)
    w = (v / (wsum[:, None] + F32(1e-6))).astype(F32)
    R, t = _kabsch(sknn, tknn, w)

    # ---- L6: fitness on device ----
    nc6 = _get_prog("fit", _prog_fitness)
    HN = NPTS // 2
    srcb = np.empty((128, 3 * HN), F32); tgtb = np.empty((128, 3 * HN), F32)
    for h in range(2):
        blk = np.transpose(src[h * HN:(h + 1) * HN], (1, 0)).reshape(3 * HN)
        srcb[h::2, :] = blk[None, :]
        blkt = np.transpose(tgt[h * HN:(h + 1) * HN], (1, 0)).reshape(3 * HN)
        tgtb[h::2, :] = blkt[None, :]
    in_maps = []
    for c in range(NCORES):
        r12 = np.zeros((128, 12), F32)
        for s in range(SPC):
            seed = c * SPC + s
            row = np.concatenate([
                [R[seed, 0, 0], R[seed, 0, 1], R[seed, 0, 2], t[seed, 0]],
                [R[seed, 1, 0], R[seed, 1, 1], R[seed, 1, 2], t[seed, 1]],
                [R[seed, 2, 0], R[seed, 2, 1], R[seed, 2, 2], t[seed, 2]]]).astype(F32)
            r12[2 * s, :] = row
            r12[2 * s + 1, :] = row
        in_maps.append({"srcb": srcb, "tgtb": tgtb, "r12": r12})
    for _try in range(4):
        res = _run(nc6, in_maps)
        _cnts = np.concatenate([res[c]["cnt"][:, 0] for c in range(NCORES)])
        if (_cnts == np.round(_cnts)).all() and (_cnts >= 0).all() and (_cnts <= NPTS).all():
            break
    fitness = np.zeros(SEEDS, np.int64)
    for c in range(NCORES):
        cc = res[c]["cnt"][:, 0]
        for s in range(SPC):
            fitness[c * SPC + s] = int(cc[2 * s]) + int(cc[2 * s + 1])

    import os
    if os.environ.get("KDBG"):
        np.save('/tmp/dbg_fit.npy', fitness)
        np.save('/tmp/dbg_R.npy', R); np.save('/tmp/dbg_t.npy', t)
        np.save('/tmp/dbg_sknn.npy', sknn); np.save('/tmp/dbg_knn.npy', knn)
    best = int(np.argmax(fitness))
    T = np.zeros((1, 4, 4), F32)
    T[0, :3, :3] = R[best]
    T[0, :3, 3] = t[best]
    T[0, 3, 3] = 1.0
    return T



# revision 8
# speedup vs baseline: 4.1138x; 4.1138x over previous
"""Trainium2 Bass kernel for nn_HCF_module (SC2 NMS/registration pipeline).

Sharding: 512 seeds split across 8 NeuronCores (64 seeds/core, keypoints
replicated). Three device launches per call, each dispatched through an
AOT-compiled (cached) shard_map executable to avoid per-launch retrace:
  L1 topk:  per-seed top-200 extraction over SC2 rows (DVE max/max_index/
            match_replace rounds on two 1024-wide halves; host merges with
            exact jax tie order + rare full-row fallback)
  L2 filt:  all four hierarchical filter stages (200->100->50->25->12) in one
            launch. Gather-free: per-seed alive-mask + rank over the fixed
            200 slots; selection keys sc2*256+(255-pos) are exact small
            integers in f32, so device ranking reproduces lax.top_k tie
            semantics bit-exactly.
  L3 fit:   fitness inlier counts; keypoints shipped once (4 rows) and
            broadcast to 128 partitions on-device via doubling SBUF DMAs.
Host glue: index gathers, final k=12 power iteration + Kabsch (validated f32
emulation), argmax.
"""
import numpy as np

F32 = np.float32
T2 = F32(0.1) * F32(0.1)            # 0.010000000707...
TWO_T2 = F32(2.0) * T2
T4 = T2 * T2
NCORES = 8
SEEDS = 512
SPC = SEEDS // NCORES               # seeds per core
NPTS = 2048
K1 = 200

_programs = {}
_launch_wall = []


def _mk_bass():
    import concourse.bass as bass
    return bass.Bass("TRN2", target_bir_lowering=False)


# --------------------------- device programs -----------------------------

def _prog_topk():
    """[128, 1024] f32 (row 2s+h = seed s, half h) -> top-136 values+idx per half.
    Outputs ym [128,136] f32, yi [128,136] uint32 (local idx in half)."""
    import concourse.mybir as mybir
    nc = _mk_bass()
    P, HN, R = 128, NPTS // 2, 17
    x = nc.dram_tensor("x", [P, HN], mybir.dt.float32, kind="ExternalInput")
    ym = nc.dram_tensor("ym", [P, 8 * R], mybir.dt.float32, kind="ExternalOutput")
    yi = nc.dram_tensor("yi", [P, 8 * R], mybir.dt.uint32, kind="ExternalOutput")
    ctx = nc.ctx
    t = ctx.enter_context(nc.sbuf_tensor([P, HN], mybir.dt.float32))
    m8 = ctx.enter_context(nc.sbuf_tensor([P, 8 * R], mybir.dt.float32))
    i8 = ctx.enter_context(nc.sbuf_tensor([P, 8 * R], mybir.dt.uint32))
    dma_sem = ctx.enter_context(nc.semaphore())
    vsem = ctx.enter_context(nc.semaphore())
    with nc.Block() as block:
        @block.gpsimd
        def _(gpsimd):
            gpsimd.dma_start(t[:, :], x[:, :]).then_inc(dma_sem, 16)
            gpsimd.wait_ge(vsem, 3 * R)
            gpsimd.dma_start(ym[:, :], m8[:, :]).then_inc(dma_sem, 16)
            gpsimd.dma_start(yi[:, :], i8[:, :]).then_inc(dma_sem, 16)
            gpsimd.wait_ge(dma_sem, 48)

        @block.vector
        def _(vector):
            vector.wait_ge(dma_sem, 16)
            n = 0
            for r in range(R):
                sl = slice(r * 8, (r + 1) * 8)
                nc.vector.max(out=m8[:, sl], in_=t[:, :]).then_inc(vsem, 1)
                n += 1
                vector.wait_ge(vsem, n)
                nc.vector.max_index(out=i8[:, sl], in_max=m8[:, sl],
                                    in_values=t[:, :]).then_inc(vsem, 1)
                n += 1
                nc.vector.match_replace(out=t[:, :], in_to_replace=m8[:, sl],
                                        in_values=t[:, :], imm_value=-1e30).then_inc(vsem, 1)
                n += 1
                vector.wait_ge(vsem, n)
    return nc


def _prog_filt():
    """gx,gy [SPC, 600] f32 (c-major: x|y|z rows of the 200 knn points) ->
    rank [SPC, 200] f32: final filter rank (survivors have rank < 12,
    ordered by rank = reference's final array order)."""
    import concourse.mybir as mybir
    from concourse.alu_op_type import AluOpType as OP
    nc = _mk_bass()
    P, K, B = SPC, K1, 20
    NB = K // B
    dt = mybir.dt.float32
    gx = nc.dram_tensor("gx", [P, 3 * K], dt, kind="ExternalInput")
    gy = nc.dram_tensor("gy", [P, 3 * K], dt, kind="ExternalInput")
    outr = nc.dram_tensor("rank", [P, K], dt, kind="ExternalOutput")
    ctx = nc.ctx

    def sb(name, shape):
        return ctx.enter_context(nc.sbuf_tensor(name, shape, dt))

    tx = sb("tx", [P, 3 * K]); ty = sb("ty", [P, 3 * K])
    dxs = sb("dxs", [P, B * 3 * K])
    d2a = sb("d2a", [P, B * K]); d2b = sb("d2b", [P, B * K])
    qb = sb("qb", [P, B * K]); pdb = sb("pdb", [P, B * K])
    hardb = sb("hardb", [P, B * K]); scrb = sb("scrb", [P, B * K])
    mask = sb("mask", [P, K]); pos = sb("pos", [P, K])
    rnk = sb("rnk", [P, K]); sc2 = sb("sc2", [P, K])
    key = sb("key", [P, K]); h0m = sb("h0m", [P, K]); ind0 = sb("ind0", [P, K])
    ta = sb("ta", [P, K]); tb = sb("tb", [P, K])
    tc = sb("tc", [P, K]); td = sb("td", [P, K])
    ones = sb("ones", [P, K]); neg = sb("neg", [P, K]); part = sb("part", [P, K])
    cxs = sb("cxs", [P, 8])
    dma_sem = ctx.enter_context(nc.semaphore())
    vsem = ctx.enter_context(nc.semaphore())

    with nc.Block() as block:
        @block.gpsimd
        def _(g):
            g.dma_start(tx[:, :], gx[:, :]).then_inc(dma_sem, 16)
            g.dma_start(ty[:, :], gy[:, :]).then_inc(dma_sem, 16)
            g.wait_ge(vsem, 1)
            g.dma_start(outr[:, :], rnk[:, :]).then_inc(dma_sem, 16)
            g.wait_ge(dma_sem, 48)

        @block.vector
        def _(v):
            V = nc.vector
            v.wait_ge(dma_sem, 32)
            tx3 = tx[:, :].rearrange("p (c k) -> p c k", c=3)
            ty3 = ty[:, :].rearrange("p (c k) -> p c k", c=3)
            # pos = iota 0..K-1 (f32, exact) via prefix scan of ones
            V.memset(ones[:, :], 1.0)
            V.memset(neg[:, :], -1e30)
            V.tensor_tensor_scan(pos[:, :], ones[:, :], neg[:, :], -1.0,
                                 OP.add, OP.max)
            V.memset(mask[:, :], 1.0)
            last = None
            for st, new_k in enumerate((100, 50, 25, 12)):
                # ---- h0m: masked hard-bit row of the rank-0 (seed) element ----
                if st == 0:
                    cax = [tx3[:, c, 0:1] for c in range(3)]
                    cbx = [ty3[:, c, 0:1] for c in range(3)]
                else:
                    V.tensor_scalar(ind0[:, :], pos[:, :], 0.0, None, OP.is_equal)
                    for c in range(3):
                        V.tensor_tensor(out=ta[:, :], in0=tx3[:, c, :],
                                        in1=ind0[:, :], op=OP.mult)
                        V.tensor_reduce(out=cxs[:, c:c + 1], in_=ta[:, :],
                                        axis=mybir.AxisListType.X, op=OP.add)
                        V.tensor_tensor(out=ta[:, :], in0=ty3[:, c, :],
                                        in1=ind0[:, :], op=OP.mult)
                        V.tensor_reduce(out=cxs[:, 4 + c:5 + c], in_=ta[:, :],
                                        axis=mybir.AxisListType.X, op=OP.add)
                    cax = [cxs[:, c:c + 1] for c in range(3)]
                    cbx = [cxs[:, 4 + c:5 + c] for c in range(3)]
                for (t3, cs, dst) in ((tx3, cax, ta), (ty3, cbx, tb)):
                    for c in range(3):
                        V.tensor_scalar(td[:, :], t3[:, c, :], cs[c], None,
                                        OP.subtract)
                        if c == 0:
                            V.tensor_tensor(out=dst[:, :], in0=td[:, :],
                                            in1=td[:, :], op=OP.mult)
                        else:
                            V.tensor_tensor(out=tc[:, :], in0=td[:, :],
                                            in1=td[:, :], op=OP.mult)
                            V.tensor_tensor(out=dst[:, :], in0=dst[:, :],
                                            in1=tc[:, :], op=OP.add)
                V.tensor_tensor(out=tc[:, :], in0=ta[:, :], in1=tb[:, :], op=OP.add)
                V.tensor_tensor(out=td[:, :], in0=ta[:, :], in1=tb[:, :], op=OP.subtract)
                V.tensor_tensor(out=td[:, :], in0=td[:, :], in1=td[:, :], op=OP.mult)
                V.tensor_scalar(ta[:, :], tc[:, :], float(TWO_T2), float(T4),
                                OP.mult, OP.subtract)
                V.tensor_tensor(out=h0m[:, :], in0=td[:, :], in1=ta[:, :], op=OP.is_lt)
                V.tensor_scalar(tb[:, :], tc[:, :], float(T2), None, OP.is_lt)
                V.tensor_tensor(out=h0m[:, :], in0=h0m[:, :], in1=tb[:, :], op=OP.max)
                V.tensor_tensor(out=h0m[:, :], in0=h0m[:, :], in1=mask[:, :], op=OP.mult)
                # ---- sc2[j] = sum_i h0m[i] * hard[i,j] (blocked over i) ----
                V.memset(sc2[:, :], 0.0)
                for bi in range(NB):
                    a0 = bi * B
                    for (src_t, dst) in ((tx, d2a), (ty, d2b)):
                        v3 = src_t[:, :].rearrange("p (c k) -> p c k", c=3)
                        rows4 = v3.unsqueeze(1).to_broadcast([P, B, 3, K])
                        cols4 = v3[:, :, a0:a0 + B].transpose([0, 2, 1]).unsqueeze(3).to_broadcast([P, B, 3, K])
                        dx4 = dxs[:, :].rearrange("p (a c k) -> p a c k", a=B, c=3)
                        V.tensor_tensor(out=dx4, in0=rows4, in1=cols4, op=OP.subtract)
                        V.tensor_tensor(out=dxs[:, :], in0=dxs[:, :], in1=dxs[:, :], op=OP.mult)
                        d2v = dst[:, :].rearrange("p (a k) -> p a k", a=B)
                        V.tensor_tensor(out=d2v, in0=dx4[:, :, 0, :], in1=dx4[:, :, 1, :], op=OP.add)
                        V.tensor_tensor(out=d2v, in0=d2v, in1=dx4[:, :, 2, :], op=OP.add)
                    V.tensor_tensor(out=qb[:, :], in0=d2a[:, :], in1=d2b[:, :], op=OP.add)
                    V.tensor_tensor(out=pdb[:, :], in0=d2a[:, :], in1=d2b[:, :], op=OP.subtract)
                    V.tensor_tensor(out=pdb[:, :], in0=pdb[:, :], in1=pdb[:, :], op=OP.mult)
                    V.tensor_scalar(scrb[:, :], qb[:, :], float(TWO_T2), float(T4),
                                    OP.mult, OP.subtract)
                    V.tensor_tensor(out=hardb[:, :], in0=pdb[:, :], in1=scrb[:, :], op=OP.is_lt)
                    V.tensor_scalar(scrb[:, :], qb[:, :], float(T2), None, OP.is_lt)
                    V.tensor_tensor(out=hardb[:, :], in0=hardb[:, :], in1=scrb[:, :], op=OP.max)
                    hv = hardb[:, :].rearrange("p (a k) -> p a k", a=B)
                    h0c = h0m[:, a0:a0 + B].unsqueeze(2).to_broadcast([P, B, K])
                    V.tensor_tensor(out=hv, in0=hv, in1=h0c, op=OP.mult)
                    V.tensor_reduce(out=part[:, :], in_=hv.transpose([0, 2, 1]),
                                    axis=mybir.AxisListType.X, op=OP.add)
                    V.tensor_tensor(out=sc2[:, :], in0=sc2[:, :], in1=part[:, :], op=OP.add)
                # ---- selection key (exact integers; dead slots -> -1e30) ----
                V.tensor_scalar(key[:, :], sc2[:, :], 256.0, 255.0, OP.mult, OP.add)
                V.tensor_tensor(out=key[:, :], in0=key[:, :], in1=pos[:, :], op=OP.subtract)
                V.tensor_tensor(out=ta[:, :], in0=key[:, :], in1=mask[:, :], op=OP.mult)
                V.tensor_scalar(tb[:, :], mask[:, :], 1.0, None, OP.subtract)
                V.scalar_tensor_tensor(out=key[:, :], in0=tb[:, :], scalar=1e30,
                                       in1=ta[:, :], op0=OP.mult, op1=OP.add)
                # ---- rnk[j] = #(key_i > key_j) ----
                V.memset(rnk[:, :], 0.0)
                for bi in range(NB):
                    a0 = bi * B
                    rowv = key[:, a0:a0 + B].unsqueeze(2).to_broadcast([P, B, K])
                    colv = key[:, :].unsqueeze(1).to_broadcast([P, B, K])
                    cb = hardb[:, :].rearrange("p (a k) -> p a k", a=B)
                    V.tensor_tensor(out=cb, in0=rowv, in1=colv, op=OP.is_gt)
                    V.tensor_reduce(out=part[:, :], in_=cb.transpose([0, 2, 1]),
                                    axis=mybir.AxisListType.X, op=OP.add)
                    last = V.tensor_tensor(out=rnk[:, :], in0=rnk[:, :],
                                           in1=part[:, :], op=OP.add)
                # ---- select ----
                if new_k != 12:
                    V.tensor_scalar(mask[:, :], rnk[:, :], float(new_k), None, OP.is_lt)
                    V.tensor_copy(pos[:, :], rnk[:, :])
            last.then_inc(vsem, 1)
    return nc


def _prog_fit():
    """kp [4, 3*1024] (rows: src h0, src h1, tgt h0, tgt h1; c-major),
    r12 [128, 12] (row 2s+h = seed s) -> cnt [128, 1] inlier counts."""
    import concourse.mybir as mybir
    from concourse.alu_op_type import AluOpType as OP
    nc = _mk_bass()
    P, HN = 128, NPTS // 2
    kp = nc.dram_tensor("kp", [4, 3 * HN], mybir.dt.float32, kind="ExternalInput")
    r12 = nc.dram_tensor("r12", [P, 12], mybir.dt.float32, kind="ExternalInput")
    cnt = nc.dram_tensor("cnt", [P, 1], mybir.dt.float32, kind="ExternalOutput")
    ctx = nc.ctx
    ts_ = ctx.enter_context(nc.sbuf_tensor([P, 3 * HN], mybir.dt.float32))
    tt_ = ctx.enter_context(nc.sbuf_tensor([P, 3 * HN], mybir.dt.float32))
    tr = ctx.enter_context(nc.sbuf_tensor([P, 12], mybir.dt.float32))
    acc = ctx.enter_context(nc.sbuf_tensor([P, HN], mybir.dt.float32))
    dc = ctx.enter_context(nc.sbuf_tensor([P, 3 * HN], mybir.dt.float32))
    l2s = ctx.enter_context(nc.sbuf_tensor([P, HN], mybir.dt.float32))
    sq = ctx.enter_context(nc.sbuf_tensor([P, HN], mybir.dt.float32))
    ccol = ctx.enter_context(nc.sbuf_tensor([P, 1], mybir.dt.float32))
    dma_sem = ctx.enter_context(nc.semaphore())
    vsem = ctx.enter_context(nc.semaphore())
    # broadcast doubling steps: partitions 2 -> 4 -> ... -> 128
    steps = [2, 4, 8, 16, 32, 64]
    dma_total = 48 + 32 * len(steps)

    with nc.Block() as block:
        @block.gpsimd
        def _(g):
            g.dma_start(ts_[0:2, :], kp[0:2, :]).then_inc(dma_sem, 16)
            g.dma_start(tt_[0:2, :], kp[2:4, :]).then_inc(dma_sem, 16)
            g.dma_start(tr[:, :], r12[:, :]).then_inc(dma_sem, 16)
            n = 48
            g.wait_ge(dma_sem, n)  # all three input DMAs landed
            for m in steps:
                g.dma_start(ts_[m:2 * m, :], ts_[0:m, :]).then_inc(dma_sem, 16)
                g.dma_start(tt_[m:2 * m, :], tt_[0:m, :]).then_inc(dma_sem, 16)
                n += 32
                g.wait_ge(dma_sem, n)
            g.wait_ge(vsem, 1)
            g.dma_start(cnt[:, :], ccol[:, :]).then_inc(dma_sem, 16)
            g.wait_ge(dma_sem, dma_total + 16)

        @block.vector
        def _(vector):
            V = nc.vector
            vector.wait_ge(dma_sem, dma_total)
            xv = ts_[:, :].rearrange("p (c b) -> p c b", c=3)
            yvv = tt_[:, :].rearrange("p (c b) -> p c b", c=3)
            dv = dc[:, :].rearrange("p (c b) -> p c b", c=3)
            for c in range(3):
                V.tensor_scalar(acc[:, :], xv[:, 0, :], tr[:, 4 * c:4 * c + 1],
                                tr[:, 4 * c + 3:4 * c + 4], OP.mult, OP.add)
                for j in (1, 2):
                    V.scalar_tensor_tensor(
                        out=acc[:, :], in0=xv[:, j, :],
                        scalar=tr[:, 4 * c + j:4 * c + j + 1],
                        in1=acc[:, :], op0=OP.mult, op1=OP.add)
                V.tensor_tensor(out=dv[:, c, :], in0=acc[:, :], in1=yvv[:, c, :],
                                op=OP.subtract)
            V.tensor_tensor(out=l2s[:, :], in0=dv[:, 0, :], in1=dv[:, 0, :], op=OP.mult)
            V.tensor_tensor(out=sq[:, :], in0=dv[:, 1, :], in1=dv[:, 1, :], op=OP.mult)
            V.tensor_tensor(out=l2s[:, :], in0=l2s[:, :], in1=sq[:, :], op=OP.add)
            V.tensor_tensor(out=sq[:, :], in0=dv[:, 2, :], in1=dv[:, 2, :], op=OP.mult)
            V.tensor_tensor(out=l2s[:, :], in0=l2s[:, :], in1=sq[:, :], op=OP.add)
            V.tensor_scalar(sq[:, :], l2s[:, :], float(T2), None, OP.is_lt)
            V.tensor_reduce(out=ccol[:, :], in_=sq[:, :],
                            axis=mybir.AxisListType.X, op=OP.add).then_inc(vsem, 1)
    return nc


# --------------------------- cached AOT dispatch --------------------------

class _AotProg:
    """AOT-compiled SPMD dispatch of a Bass program on cores 0..7.

    Mirrors bass_utils.run_bass_kernel_spmd's axon path (bass2jax) but
    builds the jit-compiled shard_map executable once and reuses it, so a
    warm launch is a single PJRT dispatch instead of retrace+relower."""

    def __init__(self, nc):
        import jax
        from jax.sharding import Mesh, PartitionSpec
        from jax.experimental.shard_map import shard_map
        import concourse.mybir as mybir
        from concourse import bass2jax
        bass2jax.install_neuronx_cc_hook()
        self.nc = nc
        part_name = nc.partition_id_tensor.name if nc.partition_id_tensor else None
        assert nc.dbg_addr is None
        in_names, out_names, out_avals = [], [], []
        for alloc in nc.m.functions[0].allocations:
            if not isinstance(alloc, mybir.MemoryLocationSet):
                continue
            name = alloc.memorylocations[0].name
            if alloc.kind == "ExternalInput":
                if name != part_name:
                    in_names.append(name)
            elif alloc.kind == "ExternalOutput":
                out_names.append(name)
                out_avals.append(jax.core.ShapedArray(
                    tuple(alloc.tensor_shape), mybir.dt.np(alloc.dtype)))
        self.in_names, self.out_names, self.out_avals = in_names, out_names, out_avals
        n_params, n_outs = len(in_names), len(out_avals)
        all_names = in_names + out_names + ([part_name] if part_name else [])

        def _body(*args):
            operands = list(args)
            if part_name is not None:
                operands.append(bass2jax.partition_id_tensor())
            return tuple(bass2jax._bass_exec_p.bind(
                *operands, out_avals=tuple(out_avals), in_names=tuple(all_names),
                out_names=tuple(out_names), lowering_input_output_aliases=(),
                sim_require_finite=True, sim_require_nnan=True, nc=nc))

        devices = jax.devices()[:NCORES]
        assert len(devices) == NCORES
        mesh = Mesh(np.asarray(devices), ("core",))
        self._fn = jax.jit(
            shard_map(_body, mesh=mesh,
                      in_specs=(PartitionSpec("core"),) * (n_params + n_outs),
                      out_specs=(PartitionSpec("core"),) * n_outs,
                      check_rep=False),
            donate_argnums=tuple(range(n_params, n_params + n_outs)),
            keep_unused=True)

    def __call__(self, **inputs):
        """inputs: name -> concat array [8*d0, ...]. Returns name -> concat."""
        import time
        args = [np.ascontiguousarray(inputs[n]) for n in self.in_names]
        last = None
        for _attempt in range(3):
            try:
                zeros = [np.zeros((NCORES * av.shape[0], *av.shape[1:]), av.dtype)
                         for av in self.out_avals]
                t0 = time.time()
                outs = self._fn(*args, *zeros)
                res = {n: np.asarray(o) for n, o in zip(self.out_names, outs)}
                _launch_wall.append(time.time() - t0)
                return res
            except Exception as e:  # transient device errors: retry
                last = e
        raise last


def _get_prog(key, builder):
    if key not in _programs:
        _programs[key] = _AotProg(builder())
    return _programs[key]


# ---------------- host-side math (validated f32 device-grade model) -------------

def _topk_host(vals, kk):
    return np.argsort(-vals, axis=-1, kind='stable')[..., :kk]


def _recip(x):
    return (np.float64(1.0) / x.astype(np.float64)).astype(F32)


def _sqrt32(x):
    return np.sqrt(x.astype(np.float64)).astype(F32)


def _cross3(a, b):
    c0 = (a[..., 1] * b[..., 2]).astype(F32) - (a[..., 2] * b[..., 1]).astype(F32)
    c1 = (a[..., 2] * b[..., 0]).astype(F32) - (a[..., 0] * b[..., 2]).astype(F32)
    c2 = (a[..., 0] * b[..., 1]).astype(F32) - (a[..., 1] * b[..., 0]).astype(F32)
    return np.stack([c0.astype(F32), c1.astype(F32), c2.astype(F32)], -1)


def _eig3(K):
    S = K.shape[0]
    qq = ((K[:, 0, 0] + K[:, 1, 1]).astype(F32) + K[:, 2, 2]).astype(F32) * F32(1 / 3)
    qq = qq.astype(F32)
    K00 = (K[:, 0, 0] - qq).astype(F32); K11 = (K[:, 1, 1] - qq).astype(F32); K22 = (K[:, 2, 2] - qq).astype(F32)
    p1 = ((K[:, 0, 1] ** 2).astype(F32) + (K[:, 0, 2] ** 2).astype(F32) + (K[:, 1, 2] ** 2).astype(F32)).astype(F32)
    p2 = ((K00 ** 2).astype(F32) + (K11 ** 2).astype(F32) + (K22 ** 2).astype(F32) + (F32(2) * p1).astype(F32)).astype(F32)
    p = _sqrt32((p2 * F32(1 / 6)).astype(F32))
    rp = _recip(np.maximum(p, F32(1e-30)))
    B00 = (K00 * rp).astype(F32); B11 = (K11 * rp).astype(F32); B22 = (K22 * rp).astype(F32)
    B01 = (K[:, 0, 1] * rp).astype(F32); B02 = (K[:, 0, 2] * rp).astype(F32); B12 = (K[:, 1, 2] * rp).astype(F32)
    detB = (B00 * ((B11 * B22).astype(F32) - (B12 * B12).astype(F32)).astype(F32)).astype(F32) \
        - (B01 * ((B01 * B22).astype(F32) - (B12 * B02).astype(F32)).astype(F32)).astype(F32) \
        + (B02 * ((B01 * B12).astype(F32) - (B11 * B02).astype(F32)).astype(F32)).astype(F32)
    r = np.clip((detB.astype(F32) * F32(0.5)).astype(F32), F32(-1), F32(1))
    c = np.ones(S, F32)
    for _ in range(6):
        f = ((F32(4) * c * c * c).astype(F32) - (F32(3) * c).astype(F32) - r).astype(F32)
        fp = ((F32(12) * c * c).astype(F32) - F32(3)).astype(F32)
        c = np.clip((c - (f * _recip(np.maximum(fp, F32(1e-6)))).astype(F32)).astype(F32), F32(0.5), F32(1.0))
    s_ = _sqrt32(np.maximum((F32(1) - (c * c).astype(F32)).astype(F32), F32(0)))
    lam1 = (qq + (F32(2) * p * c).astype(F32)).astype(F32)
    cmid = ((F32(-0.5) * c).astype(F32) + (F32(np.sqrt(3) / 2) * s_).astype(F32)).astype(F32)
    lam2 = (qq + (F32(2) * p * cmid).astype(F32)).astype(F32)
    return lam1, lam2


def _eigvec(K, lam):
    A = K.astype(F32).copy()
    for i in range(3):
        A[:, i, i] = (A[:, i, i] - lam).astype(F32)
    r0, r1, r2 = A[:, 0, :], A[:, 1, :], A[:, 2, :]
    c1 = _cross3(r0, r1); c2 = _cross3(r1, r2); c3 = _cross3(r2, r0)
    n1 = (c1 ** 2).sum(-1).astype(F32); n2 = (c2 ** 2).sum(-1).astype(F32); n3 = (c3 ** 2).sum(-1).astype(F32)
    a1 = (n1 >= n2) & (n1 >= n3); a2 = (~a1) & (n2 >= n3); a3 = ~(a1 | a2)
    u = (c1 * a1[:, None] + c2 * a2[:, None] + c3 * a3[:, None]).astype(F32)
    n = (u ** 2).sum(-1).astype(F32)
    return (u * _recip(_sqrt32(np.maximum(n, F32(1e-38))))[:, None]).astype(F32)


def _kabsch(A, B, w):
    S = A.shape[0]
    wsum = w.sum(axis=1, dtype=np.float32)
    rws = _recip((wsum + F32(1e-6)).astype(F32))
    wA = (A * w[:, :, None]).astype(F32); wB = (B * w[:, :, None]).astype(F32)
    cA = (wA.sum(axis=1, dtype=np.float32) * rws[:, None]).astype(F32)
    cB = (wB.sum(axis=1, dtype=np.float32) * rws[:, None]).astype(F32)
    Am = (A - cA[:, None, :]).astype(F32); Bm = (B - cB[:, None, :]).astype(F32)
    wAm = (Am * w[:, :, None]).astype(F32)
    H = np.einsum('ski,skj->sij', wAm, Bm).astype(F32)
    K = np.einsum('sij,skj->sik', H, H).astype(F32)
    lam1, lam2 = _eig3(K)
    u1 = _eigvec(K, lam1)
    u2r = _eigvec(K, lam2)
    dot = (u1 * u2r).sum(-1).astype(F32)
    u2 = (u2r - u1 * dot[:, None]).astype(F32)
    n = (u2 ** 2).sum(-1).astype(F32)
    u2 = (u2 * _recip(_sqrt32(np.maximum(n, F32(1e-38))))[:, None]).astype(F32)
    u3 = _cross3(u1, u2)
    w1 = np.einsum('ski,sk->si', H, u1).astype(F32)
    w2 = np.einsum('ski,sk->si', H, u2).astype(F32)
    v1 = (w1 * _recip(_sqrt32(np.maximum((w1 ** 2).sum(-1).astype(F32), F32(1e-38))))[:, None]).astype(F32)
    v2 = (w2 * _recip(_sqrt32(np.maximum((w2 ** 2).sum(-1).astype(F32), F32(1e-38))))[:, None]).astype(F32)
    v3 = _cross3(v1, v2)
    R = (v1[:, :, None] * u1[:, None, :] + v2[:, :, None] * u2[:, None, :]
         + v3[:, :, None] * u3[:, None, :]).astype(F32)
    t = (cB - np.einsum('sij,sj->si', R, cA).astype(F32)).astype(F32)
    return R, t


def _power_iter(M):
    S, k, _ = M.shape
    v = np.ones((S, k), F32)
    for _ in range(10):
        prod = (M * v[:, None, :]).astype(F32)
        acc = prod[:, :, 0]
        for j in range(1, k):
            acc = (acc + prod[:, :, j]).astype(F32)
        n2 = (acc * acc).astype(F32)
        s2 = n2[:, 0]
        for j in range(1, k):
            s2 = (s2 + n2[:, j]).astype(F32)
        nn_ = _sqrt32(s2)
        v = (acc * _recip((nn_ + F32(1e-6)).astype(F32))[:, None]).astype(F32)
    return v


def _pdist2(pts):
    d = (pts[:, :, None, :] - pts[:, None, :, :]).astype(F32)
    sq = (d * d).astype(F32)
    return ((sq[..., 0] + sq[..., 1]).astype(F32) + sq[..., 2]).astype(F32)


def kernel(SC2_measure, src_keypts, tgt_keypts):
    _launch_wall.clear()
    SC2 = np.ascontiguousarray(SC2_measure[0], dtype=np.float32)      # [512, 2048]
    src = np.ascontiguousarray(src_keypts[0], dtype=np.float32)       # [2048, 3]
    tgt = np.ascontiguousarray(tgt_keypts[0], dtype=np.float32)

    # ---- L1: per-seed top-200 on device (rows split into 2 halves) ----
    p1 = _get_prog("topk", _prog_topk)
    HN = NPTS // 2
    xh = SC2.reshape(SEEDS * 2, HN)                                  # row 2s+h
    for _try in range(4):
        res = p1(x=xh)
        vm = res["ym"]                                               # [1024, 136]
        vi = res["yi"].astype(np.int64)
        if (vi < HN).all():
            break
    # merge halves: concat [A|B]; stable sort by value desc == jax global order
    NE = vm.shape[1]
    cand_v = np.concatenate([vm[0::2], vm[1::2]], axis=1)            # [512, 2*NE]
    cand_i = np.concatenate([vi[0::2], vi[1::2] + HN], axis=1)
    order = np.argsort(-cand_v, axis=1, kind='stable')[:, :K1]
    knn = np.take_along_axis(cand_i, order, axis=1)                  # [512, 200]
    # safety: if any seed's 200th value ties the last extracted value of a
    # half, extraction may be incomplete -> exact host fallback for that seed
    thr = np.take_along_axis(cand_v, order[:, K1 - 1:K1], axis=1)[:, 0]
    risky = (vm[0::2, NE - 1] >= thr) | (vm[1::2, NE - 1] >= thr)
    for s in np.where(risky)[0]:
        knn[s] = np.argsort(-SC2[s], kind='stable')[:K1]
    sknn = src[knn].astype(F32)                                       # [512, 200, 3]
    tknn = tgt[knn].astype(F32)

    # ---- L2: all four filter stages fused on device ----
    p2 = _get_prog("filt", _prog_filt)
    gxa = np.ascontiguousarray(np.transpose(sknn, (0, 2, 1)).reshape(SEEDS, 3 * K1))
    gya = np.ascontiguousarray(np.transpose(tknn, (0, 2, 1)).reshape(SEEDS, 3 * K1))
    for _try in range(4):
        res = p2(gx=gxa, gy=gya)
        rankf = res["rank"]                                          # [512, 200]
        ok = (rankf == np.round(rankf)).all() and (rankf >= 0).all() \
            and ((rankf < 12).sum(axis=1) == 12).all()
        if ok:
            break
    sel = np.argsort(rankf, axis=1, kind='stable')[:, :12]           # [512, 12]
    sknn = np.take_along_axis(sknn, sel[:, :, None], axis=1)         # [512, 12, 3]
    tknn = np.take_along_axis(tknn, sel[:, :, None], axis=1)

    # ---- host: local_sc, power iteration, Kabsch (validated f32 model) ----
    a2 = _pdist2(sknn); b2 = _pdist2(tknn)
    da = _sqrt32(np.maximum(a2, F32(1e-12)))
    db = _sqrt32(np.maximum(b2, F32(1e-12)))
    cross = np.abs((da - db).astype(F32)).astype(F32)
    local_sc = np.maximum(F32(1.0) - ((cross * cross).astype(F32) / T2).astype(F32), F32(0.0)).astype(F32)
    eye = np.eye(12, dtype=F32)
    M = (local_sc * (F32(1.0) - eye)[None]).astype(F32)
    v = _power_iter(M)
    wsum = v[:, 0].copy()
    for j in range(1, 12):
        wsum = (wsum + v[:, j]).astype(F32)
    w = (v / (wsum[:, None] + F32(1e-6))).astype(F32)
    R, t = _kabsch(sknn, tknn, w)

    # ---- L3: fitness on device (keypoints broadcast on-device) ----
    p3 = _get_prog("fit", _prog_fit)
    kp = np.empty((4, 3 * HN), F32)
    for h in range(2):
        kp[h] = np.transpose(src[h * HN:(h + 1) * HN], (1, 0)).reshape(3 * HN)
        kp[2 + h] = np.transpose(tgt[h * HN:(h + 1) * HN], (1, 0)).reshape(3 * HN)
    kp_all = np.tile(kp, (NCORES, 1))                                # [32, 3*HN]
    # row layout per seed: [R00 R01 R02 t0 | R10 R11 R12 t1 | R20 R21 R22 t2]
    r12 = np.concatenate([
        np.concatenate([R[:, c, :], t[:, c:c + 1]], axis=1) for c in range(3)
    ], axis=1).astype(F32)                                           # [512, 12]
    r12_all = np.repeat(r12, 2, axis=0)                              # [1024, 12] row 2s+h
    for _try in range(4):
        res = p3(kp=kp_all, r12=r12_all)
        cc = res["cnt"][:, 0]                                        # [1024]
        if (cc == np.round(cc)).all() and (cc >= 0).all() and (cc <= NPTS).all():
            break
    fitness = cc[0::2].astype(np.int64) + cc[1::2].astype(np.int64)  # [512]

    best = int(np.argmax(fitness))
    T = np.zeros((1, 4, 4), F32)
    T[0, :3, :3] = R[best]
    T[0, :3, 3] = t[best]
    T[0, 3, 3] = 1.0
    return T
